# revision 37
# baseline (speedup 1.0000x reference)
"""Trainium2 Bass kernel for AidedMultiHeadAttention.

Shapes (hardcoded): B=2, S=1024, E=1024, H=16, A=3, HD=64.

Sharding: 8 cores; core i handles batch b = i//4 and heads
h in [4*(i%4), 4*(i%4)+4)  -> 4 (b,h) "pairs" per core.

On-device layout: the attention matrix is computed TRANSPOSED,
pT[k, q] (keys on partitions, queries on the free axis).  This makes
every matmul natural (no on-device transposes):
  pT tile   = kT_ext.T @ qT_ext          (contraction over head dim d)
  mixer     = accumulated into a second PSUM bank via diagonal matmuls
  out2T     = v.T-free accumulation:  lhsT=v (k,d) -> out (d, q)
  Z (softmax denominator) = ones-row matmul over e
  final     = WoT.T @ actT  (row-parallel projection; host sums partials)

Math: with c = aid_scale, per head weights w0..w3, bias b:
  pre  = p + c*relu(w0*p + w1*a0 + w2*a1 + w3*a2 + b)
       = p + sgn*relu(|c|*(...))              (sgn = sign(c))
  attn = exp(pre)/Z  (no max subtraction; |pre| ~ 6 so exp is safe)
The per-head scalars are folded into host-prepared operands (q scaled
rows, bias row, diagonal matrices, one STT scalar), so the single SPMD
program works for every core.
"""

import numpy as np
import ml_dtypes

BF16 = ml_dtypes.bfloat16

B, S, E, H, A = 2, 1024, 1024, 16, 3
HD = E // H          # 64
SCALE = HD ** -0.5
NCORES = 8
NPAIR = 4            # (b,h) pairs per core
KT = 8               # 128-row k tiles per pair
QH = 2               # 512-col q halves
KD = HD + 1          # contraction dim incl. bias row = 65

_PROG_CACHE = {}


def _build_program():
    import concourse.bacc as bacc
    import concourse.tile as tile
    import concourse.bass as bass
    from concourse import mybir

    f32 = mybir.dt.float32
    bf16 = mybir.dt.bfloat16
    AF = mybir.ActivationFunctionType
    OP = mybir.AluOpType

    nc = bacc.Bacc("TRN2", target_bir_lowering=False, debug=False)

    # ---- DRAM I/O ----
    aid_d = nc.dram_tensor("aid_l", [128, A, KT, S], bf16, kind="ExternalInput")
    kq_d = nc.dram_tensor("kq_l", [KD, NPAIR, 3, S], bf16, kind="ExternalInput")
    vx_d = nc.dram_tensor("vx_l", [128, NPAIR, KT, HD + 1], bf16,
                          kind="ExternalInput")
    dg_d = nc.dram_tensor("dg_l", [128, NPAIR, 3, 128], bf16, kind="ExternalInput")
    wot_d = nc.dram_tensor("wot_l", [128, 2, E], bf16, kind="ExternalInput")
    par_d = nc.dram_tensor("par_l", [128, 8], f32, kind="ExternalInput")

    zs_d = nc.dram_tensor("zs_scr", [NPAIR, S], f32)
    zs2_d = nc.dram_tensor("zs2_scr", [NPAIR, S], f32)
    pre_d = nc.dram_tensor("pre_o", [NPAIR, S, S], f32, kind="ExternalOutput")
    attn_d = nc.dram_tensor("attn_o", [NPAIR, S, S], f32, kind="ExternalOutput")
    fin_d = nc.dram_tensor("fin_o", [E, S], f32, kind="ExternalOutput")

    PS = bass.MemorySpace.PSUM

    from contextlib import ExitStack

    with tile.TileContext(nc) as tc, ExitStack() as ctx:
        def pool(name, bufs, space=None):
            kw = {"space": space} if space else {}
            return ctx.enter_context(tc.tile_pool(name=name, bufs=bufs, **kw))

        aid_pool = pool("aid", 1)
        kq_pool = pool("kq", 2)
        vx_pool = pool("vx", 2)
        dg_pool = pool("dg", 2)
        wot_pool = pool("wot", 1)
        par_pool = pool("par", 1)
        u1_pool = pool("u1", 2)
        r_pool = pool("rt", 3)
        pre_pool = pool("pre", 3)
        e_pool = pool("esb", 2)
        attn_pool = pool("att", 2)
        zb_pool = pool("zb", 2)
        srow_pool = pool("srow", 2)
        o2sb_pool = pool("o2sb", 2)
        actT_pool = pool("actT", 1)
        fin_pool = pool("fin", 3)
        ppb_pool = pool("ppb", 3, PS)
        ppo_pool = pool("ppo", 1, PS)

        if True:
            # ---- one-time loads ----
            # pair-0 operands and the first k-tiles of aid first, so the PE
            # can start while the bulk of aid is still in flight.
            par_sb = par_pool.tile([128, 8], f32)
            nc.sync.dma_start(par_sb[:], par_d[:])
            aid_sb = aid_pool.tile([128, A, KT, S], bf16)
            for kt in range(KT):
                for ch in range(A):
                    nc.sync.dma_start(aid_sb[:, ch, kt], aid_d[:, ch, kt])
            wot_sb = wot_pool.tile([128, 2, E], bf16)
            nc.sync.dma_start(wot_sb[:], wot_d[:])

            actT_sb = actT_pool.tile([128, 2, S], bf16)

            for pair in range(NPAIR):
                tcol = pair // 2          # actT column
                trow = 64 * (pair % 2)    # actT row base for this pair

                # SWDGE (gpsimd) queue: don't serialize behind the big
                # aid/store traffic on the Sync HWDGE queue.
                kq_sb = kq_pool.tile([KD, 3, S], bf16, tag="kq")
                nc.gpsimd.dma_start(kq_sb[:], kq_d[:, pair])
                vx_sb = vx_pool.tile([128, KT, HD + 1], bf16, tag="vx")
                nc.gpsimd.dma_start(vx_sb[:], vx_d[:, pair])
                dg_sb = dg_pool.tile([128, 3, 128], bf16, tag="dg")
                nc.gpsimd.dma_start(dg_sb[:], dg_d[:, pair])

                e_sb = e_pool.tile([128, KT, S], bf16, tag="esb")
                o2z = ppo_pool.tile([128, S], f32, tag="o2z")

                QS = [slice(0, 512), slice(512, 1024)]
                pbs, rts = {}, {}

                u1s = {}

                def stage_front(kt):
                    # u1 = (a0 * (w1/w2)) + a1   [DVE, bf16]; two k-tiles
                    # per instruction to halve DVE dispatch/sem overhead.
                    if kt % 2 == 0:
                        u12 = u1_pool.tile([128, 2, S], bf16, tag="u1")
                        nc.vector.scalar_tensor_tensor(
                            u12[:], aid_sb[:, 0, kt:kt + 2],
                            par_sb[:, pair:pair + 1],
                            aid_sb[:, 1, kt:kt + 2], OP.mult, OP.add,
                        )
                        u1s[kt] = u12[:, 0]
                        u1s[kt + 1] = u12[:, 1]
                    u1 = u1s[kt]
                    # group 1 in bank pair: |c|*(w0*p + b + w2*u1 + w3*a2)
                    kqL = kq_sb[:, 0, kt * 128:(kt + 1) * 128]
                    pb = ppb_pool.tile([128, S], f32, tag="ppb")
                    pbs[kt] = pb
                    for qh in range(QH):
                        nc.tensor.matmul(pb[:, QS[qh]], kqL,
                                         kq_sb[:, 2, QS[qh]],
                                         start=True, stop=False)
                    for qh in range(QH):
                        nc.tensor.matmul(pb[:, QS[qh]], dg_sb[:, 0],
                                         u1[:, QS[qh]],
                                         start=False, stop=False)
                    for qh in range(QH):
                        nc.tensor.matmul(pb[:, QS[qh]], dg_sb[:, 1],
                                         aid_sb[:, 2, kt, QS[qh]],
                                         start=False, stop=True)
                    # r = relu(group 1)  [ACT] -> bf16
                    r_t = r_pool.tile([128, S], bf16, tag="rt")
                    rts[kt] = r_t
                    nc.scalar.activation(r_t[:], pb[:], AF.Relu)

                def stage_mid(kt):
                    # group 2, same banks: plain p
                    kqL = kq_sb[:, 0, kt * 128:(kt + 1) * 128]
                    pb, r_t = pbs[kt], rts[kt]
                    for qh in range(QH):
                        nc.tensor.matmul(pb[:, QS[qh]], kqL,
                                         kq_sb[:, 1, QS[qh]],
                                         start=True, stop=True)
                    # pre = sgn*r + p, fused into the PSUM->SBUF move [DVE]
                    pre_t = pre_pool.tile([128, S], f32, tag="pre")
                    nc.vector.scalar_tensor_tensor(
                        pre_t[:], r_t[:], par_sb[:, 4 + pair:5 + pair],
                        pb[:], OP.mult, OP.add)
                    nc.sync.dma_start(
                        pre_d[pair, kt * 128:(kt + 1) * 128, :], pre_t[:])
                    # e = exp(pre) [ACT] -> bf16
                    nc.scalar.activation(e_sb[:, kt], pre_t[:], AF.Exp)

                def stage_back(kt):
                    # out2T rows 0:64 += v.T-free @ e ; Z row 64 via the
                    # ones column folded into vx.
                    for qh in range(QH):
                        nc.tensor.matmul(o2z[0:HD + 1, QS[qh]], vx_sb[:, kt],
                                         e_sb[:, kt, QS[qh]],
                                         start=(kt == 0), stop=(kt == KT - 1))

                for kt in range(KT + 2):
                    if kt < KT:
                        stage_front(kt)
                    if 1 <= kt:
                        if kt - 1 < KT:
                            stage_mid(kt - 1)
                    if kt >= 2:
                        stage_back(kt - 2)

                # ---- pair tail ----
                # 1/Z: spread the Z row across 128 partitions via a DRAM
                # bounce so the (slow, iterative) reciprocal runs on 8
                # elements/lane instead of 1024 on one lane.
                zrow = srow_pool.tile([KD, S], f32, tag="zrow")
                nc.vector.tensor_copy(zrow[HD:HD + 1, :], o2z[HD:HD + 1, :])
                nc.sync.dma_start(zs_d[pair], zrow[HD:HD + 1, :])
                spread = srow_pool.tile([128, S // 128], f32, tag="spread")
                nc.sync.dma_start(
                    spread[:], zs_d[pair].rearrange("(p f) -> p f", p=128))
                rec128 = srow_pool.tile([128, S // 128], f32, tag="rec128")
                nc.vector.reciprocal(rec128[:], spread[:])
                nc.sync.dma_start(
                    zs2_d[pair].rearrange("(p f) -> p f", p=128), rec128[:])
                # partition_broadcast reads absolute partition 0, so land the
                # full recip row there and broadcast.
                srow0 = srow_pool.tile([1, S], f32, tag="srow0")
                nc.sync.dma_start(
                    srow0[0:1, :], zs2_d[pair].rearrange("(a f) -> a f", a=1))
                zb2 = zb_pool.tile([128, 2, S], f32, tag="zb")
                nc.gpsimd.partition_broadcast(zb2[:, 0], srow0[0:1, :], 128)
                nc.gpsimd.partition_broadcast(zb2[:, 1], srow0[0:1, :], 128)
                zb = zb2[:, 0]

                o2sb = o2sb_pool.tile([128, S], f32, tag="o2sb")
                nc.scalar.copy(o2sb[0:HD, :], o2z[0:HD, :])
                if trow == 0:
                    nc.vector.tensor_tensor(
                        actT_sb[0:HD, tcol], o2sb[0:HD, :], zb[0:HD, :],
                        OP.mult)
                else:
                    o2n = o2sb_pool.tile([HD, S], bf16, tag="o2n")
                    nc.vector.tensor_tensor(
                        o2n[:], o2sb[0:HD, :], zb[0:HD, :], OP.mult)
                    nc.sync.dma_start(actT_sb[trow:trow + HD, tcol], o2n[:])

                # DVE normalizes k-tiles 0-3 as two double-width ops;
                # GpSimd takes k-tiles 4-7 individually in parallel.
                for kp in (0, 2):
                    attn_t2 = attn_pool.tile([128, 2, S], f32, tag="att")
                    nc.vector.tensor_tensor(attn_t2[:], e_sb[:, kp:kp + 2],
                                            zb2[:], OP.mult)
                    nc.sync.dma_start(
                        attn_d[pair, kp * 128:(kp + 2) * 128, :].rearrange(
                            "(t p) q -> p t q", p=128), attn_t2[:])
                for kt in range(4, KT):
                    attn_t = attn_pool.tile([128, 1, S], f32, tag="att")
                    nc.gpsimd.tensor_tensor(attn_t[:, 0], e_sb[:, kt], zb[:],
                                            OP.mult)
                    nc.sync.dma_start(
                        attn_d[pair, kt * 128:(kt + 1) * 128, :],
                        attn_t[:, 0])

            # ---- row-parallel projection: finT = WoT.T @ actT ----
            for eo in range(8):
                es = slice(eo * 128, (eo + 1) * 128)
                for qh in range(QH):
                    qs = slice(qh * 512, (qh + 1) * 512)
                    fin_ps = ppb_pool.tile([128, 512], f32, tag="ppb")
                    nc.tensor.matmul(fin_ps[:], wot_sb[:, 0, es],
                                     actT_sb[:, 0, qs], start=True, stop=False)
                    nc.tensor.matmul(fin_ps[:], wot_sb[:, 1, es],
                                     actT_sb[:, 1, qs], start=False, stop=True)
                    fin_t = fin_pool.tile([128, 512], f32, tag="fin")
                    nc.scalar.copy(fin_t[:], fin_ps[:])
                    nc.sync.dma_start(fin_d[es, qs], fin_t[:])

    nc.compile()
    return nc


def get_program():
    if "nc" not in _PROG_CACHE:
        _PROG_CACHE["nc"] = _build_program()
    return _PROG_CACHE["nc"]


def make_core_inputs(ci, query, key, value, aid, mixer_w, mixer_b, Wo, bo,
                     aid_scale):
    """Build the per-core input map (numpy, host-side sharding/layout)."""
    b = ci // 4
    heads = [4 * (ci % 4) + j for j in range(NPAIR)]
    c = float(np.asarray(aid_scale).reshape(-1)[0])
    a_abs = abs(c)
    sgn = 1.0 if c >= 0 else -1.0

    q4 = query.reshape(B, H, S, HD)
    k4 = key.reshape(B, H, S, HD)
    v4 = value.reshape(B, H, S, HD)

    # aid_l[p, ch, kt, q] = aid[b, q, kt*128+p, ch]
    aid_l = np.ascontiguousarray(
        aid[b].transpose(2, 1, 0)          # (ch, k, q)
        .reshape(A, KT, 128, S)
        .transpose(2, 0, 1, 3)
    ).astype(BF16)

    kq_l = np.zeros((KD, NPAIR, 3, S), np.float32)
    vx_l = np.zeros((128, NPAIR, KT, HD + 1), np.float32)
    vx_l[:, :, :, HD] = 1.0
    dg_l = np.zeros((128, NPAIR, 3, 128), np.float32)
    par_l = np.zeros((128, 8), np.float32)

    eye = np.eye(128, dtype=np.float32)
    for j, h in enumerate(heads):
        w0, w1, w2, w3 = (float(mixer_w[h, t, 0]) for t in range(A + 1))
        bm = float(mixer_b[h, 0])
        w2s = w2 if abs(w2) > 1e-8 else 1e-8
        qT = q4[b, h].T                    # (HD, S)
        kT = k4[b, h].T
        kq_l[:HD, j, 0] = kT
        kq_l[HD, j, 0] = 1.0
        kq_l[:HD, j, 1] = SCALE * qT
        kq_l[HD, j, 1] = 0.0
        kq_l[:HD, j, 2] = (a_abs * w0 * SCALE) * qT
        kq_l[HD, j, 2] = a_abs * bm
        vx_l[:, j, :, :HD] = v4[b, h].reshape(KT, 128, HD).transpose(1, 0, 2)
        dg_l[:, j, 0] = (a_abs * w2s) * eye
        dg_l[:, j, 1] = (a_abs * w3) * eye
        dg_l[:, j, 2] = sgn * eye
        par_l[:, j] = w1 / w2s
        par_l[:, 4 + j] = sgn

    e0 = 256 * (ci % 4)
    # wot_l[p, t, e_out] = Wo[e_out, e0 + t*128 + p]
    wot_l = np.ascontiguousarray(
        Wo[:, e0:e0 + 256].T.reshape(2, 128, E).transpose(1, 0, 2)
    ).astype(BF16)

    return {
        "aid_l": aid_l,
        "kq_l": kq_l.astype(BF16),
        "vx_l": vx_l.astype(BF16),
        "dg_l": dg_l.astype(BF16),
        "wot_l": wot_l,
        "par_l": par_l,
    }


def _enable_ldw_opt():
    """Walrus's LDWEIGHTS-dedup pass is disabled by default in
    bir_verify_and_optimise; our matmul stream re-loads identical
    stationary weights constantly, so turn it on (correctness is
    covered by the caller's rel-err check)."""
    return  # walrus crashes with ldw-opt=true; keep default
    if _PROG_CACHE.get("ldw_patched"):
        return
    import concourse.bass_utils as bu

    orig = bu.run_command

    def patched(argv, **kw):
        argv = [a.replace("--enable-ldw-opt=false", "--enable-ldw-opt=true")
                if isinstance(a, str) else a for a in argv]
        return orig(argv, **kw)

    bu.run_command = patched
    _PROG_CACHE["ldw_patched"] = True


def kernel(query, key, value, aid, mixer_w, mixer_b, Wo, bo, aid_scale,
           trace=False):
    from concourse.bass_utils import run_bass_kernel_spmd

    _enable_ldw_opt()

    query = np.asarray(query, np.float32)
    key = np.asarray(key, np.float32)
    value = np.asarray(value, np.float32)
    aid = np.asarray(aid, np.float32)
    mixer_w = np.asarray(mixer_w, np.float32)
    mixer_b = np.asarray(mixer_b, np.float32)
    Wo = np.asarray(Wo, np.float32)
    bo = np.asarray(bo, np.float32)
    aid_scale = np.asarray(aid_scale, np.float32)

    nc = get_program()
    in_maps = [
        make_core_inputs(ci, query, key, value, aid, mixer_w, mixer_b, Wo,
                         bo, aid_scale)
        for ci in range(NCORES)
    ]
    res = run_bass_kernel_spmd(nc, in_maps, list(range(NCORES)), trace=trace)
    _PROG_CACHE["last_results"] = res

    out = np.empty((B, S, E), np.float32)
    attn = np.empty((B, H, S, S), np.float32)
    pre = np.empty((B, H, S, S), np.float32)
    fin_acc = [None] * B
    for ci in range(NCORES):
        b = ci // 4
        r = res.results[ci]
        for j in range(NPAIR):
            h = 4 * (ci % 4) + j
            attn[b, h] = r["attn_o"][j].T
            pre[b, h] = r["pre_o"][j].T
        fin_acc[b] = r["fin_o"] if fin_acc[b] is None else fin_acc[b] + r["fin_o"]
    for b in range(B):
        out[b] = fin_acc[b].T + bo[None, :]
    return out, attn, pre


# revision 38
# speedup vs baseline: 1.0437x; 1.0437x over previous
"""Trainium2 Bass kernel for AidedMultiHeadAttention.

Shapes (hardcoded): B=2, S=1024, E=1024, H=16, A=3, HD=64.

Sharding: 8 cores; core i handles batch b = i//4 and heads
h in [4*(i%4), 4*(i%4)+4)  -> 4 (b,h) "pairs" per core.

On-device layout: the attention matrix is computed TRANSPOSED,
pT[k, q] (keys on partitions, queries on the free axis).  This makes
every matmul natural (no on-device transposes):
  pT tile   = kT_ext.T @ qT_ext          (contraction over head dim d)
  mixer     = accumulated into a second PSUM bank via diagonal matmuls
  out2T     = v.T-free accumulation:  lhsT=v (k,d) -> out (d, q)
  Z (softmax denominator) = ones-row matmul over e
  final     = WoT.T @ actT  (row-parallel projection; host sums partials)

Math: with c = aid_scale, per head weights w0..w3, bias b:
  pre  = p + c*relu(w0*p + w1*a0 + w2*a1 + w3*a2 + b)
       = p + sgn*relu(|c|*(...))              (sgn = sign(c))
  attn = exp(pre)/Z  (no max subtraction; |pre| ~ 6 so exp is safe)
The per-head scalars are folded into host-prepared operands (q scaled
rows, bias row, diagonal matrices, one STT scalar), so the single SPMD
program works for every core.
"""

import numpy as np
import ml_dtypes

BF16 = ml_dtypes.bfloat16

B, S, E, H, A = 2, 1024, 1024, 16, 3
HD = E // H          # 64
SCALE = HD ** -0.5
NCORES = 8
NPAIR = 4            # (b,h) pairs per core
KT = 8               # 128-row k tiles per pair
QH = 2               # 512-col q halves
KD = HD + 1          # contraction dim incl. bias row = 65

_PROG_CACHE = {}


def _build_program():
    import concourse.bacc as bacc
    import concourse.tile as tile
    import concourse.bass as bass
    from concourse import mybir

    f32 = mybir.dt.float32
    bf16 = mybir.dt.bfloat16
    AF = mybir.ActivationFunctionType
    OP = mybir.AluOpType

    nc = bacc.Bacc("TRN2", target_bir_lowering=False, debug=False)

    # ---- DRAM I/O ----
    aid_d = nc.dram_tensor("aid_l", [128, A, KT, S], bf16, kind="ExternalInput")
    kq_d = nc.dram_tensor("kq_l", [KD, NPAIR, 3, S], bf16, kind="ExternalInput")
    vx_d = nc.dram_tensor("vx_l", [128, NPAIR, KT, HD + 1], bf16,
                          kind="ExternalInput")
    dg_d = nc.dram_tensor("dg_l", [128, NPAIR, 3, 128], bf16, kind="ExternalInput")
    wot_d = nc.dram_tensor("wot_l", [128, 2, E], bf16, kind="ExternalInput")
    par_d = nc.dram_tensor("par_l", [128, 8], f32, kind="ExternalInput")

    zs_d = nc.dram_tensor("zs_scr", [NPAIR, S], f32)
    zs2_d = nc.dram_tensor("zs2_scr", [NPAIR, S], f32)
    pre_d = nc.dram_tensor("pre_o", [NPAIR, S, S], f32, kind="ExternalOutput")
    attn_d = nc.dram_tensor("attn_o", [NPAIR, S, S], f32, kind="ExternalOutput")
    fin_d = nc.dram_tensor("fin_o", [E, S], f32, kind="ExternalOutput")

    PS = bass.MemorySpace.PSUM

    from contextlib import ExitStack

    with tile.TileContext(nc) as tc, ExitStack() as ctx:
        def pool(name, bufs, space=None):
            kw = {"space": space} if space else {}
            return ctx.enter_context(tc.tile_pool(name=name, bufs=bufs, **kw))

        aid_pool = pool("aid", 1)
        kq_pool = pool("kq", 2)
        vx_pool = pool("vx", 2)
        dg_pool = pool("dg", 2)
        wot_pool = pool("wot", 1)
        par_pool = pool("par", 1)
        u1_pool = pool("u1", 3)
        r_pool = pool("rt", 3)
        pre_pool = pool("pre", 4)
        e_pool = pool("esb", 2)
        attn_pool = pool("att", 4)
        zb_pool = pool("zb", 2)
        srow_pool = pool("srow", 2)
        o2sb_pool = pool("o2sb", 2)
        actT_pool = pool("actT", 1)
        fin_pool = pool("fin", 3)
        ppb_pool = pool("ppb", 3, PS)
        ppo_pool = pool("ppo", 1, PS)

        if True:
            # ---- one-time loads ----
            # pair-0 operands and the first k-tiles of aid first, so the PE
            # can start while the bulk of aid is still in flight.
            par_sb = par_pool.tile([128, 8], f32)
            nc.sync.dma_start(par_sb[:], par_d[:])
            aid_sb = aid_pool.tile([128, A, KT, S], bf16)
            for kt in range(KT):
                for ch in range(A):
                    nc.sync.dma_start(aid_sb[:, ch, kt], aid_d[:, ch, kt])
            wot_sb = wot_pool.tile([128, 2, E], bf16)
            nc.sync.dma_start(wot_sb[:], wot_d[:])

            actT_sb = actT_pool.tile([128, 2, S], bf16)

            for pair in range(NPAIR):
                tcol = pair // 2          # actT column
                trow = 64 * (pair % 2)    # actT row base for this pair

                # SWDGE (gpsimd) queue: don't serialize behind the big
                # aid/store traffic on the Sync HWDGE queue.
                kq_sb = kq_pool.tile([KD, 3, S], bf16, tag="kq")
                nc.gpsimd.dma_start(kq_sb[:], kq_d[:, pair])
                vx_sb = vx_pool.tile([128, KT, HD + 1], bf16, tag="vx")
                nc.gpsimd.dma_start(vx_sb[:], vx_d[:, pair])
                dg_sb = dg_pool.tile([128, 3, 128], bf16, tag="dg")
                nc.gpsimd.dma_start(dg_sb[:], dg_d[:, pair])

                e_sb = e_pool.tile([128, KT, S], bf16, tag="esb")
                o2z = ppo_pool.tile([128, S], f32, tag="o2z")

                QS = [slice(0, 512), slice(512, 1024)]
                pbs, rts = {}, {}

                def stage_front(kt):
                    # u1 = (a0 * (w1/w2)) + a1   [DVE, bf16]
                    u1 = u1_pool.tile([128, S], bf16, tag="u1")
                    nc.vector.scalar_tensor_tensor(
                        u1[:], aid_sb[:, 0, kt], par_sb[:, pair:pair + 1],
                        aid_sb[:, 1, kt], OP.mult, OP.add,
                    )
                    # group 1 in bank pair: |c|*(w0*p + b + w2*u1 + w3*a2)
                    kqL = kq_sb[:, 0, kt * 128:(kt + 1) * 128]
                    pb = ppb_pool.tile([128, S], f32, tag="ppb")
                    pbs[kt] = pb
                    for qh in range(QH):
                        nc.tensor.matmul(pb[:, QS[qh]], kqL,
                                         kq_sb[:, 2, QS[qh]],
                                         start=True, stop=False)
                    for qh in range(QH):
                        nc.tensor.matmul(pb[:, QS[qh]], dg_sb[:, 0],
                                         u1[:, QS[qh]],
                                         start=False, stop=False)
                    for qh in range(QH):
                        nc.tensor.matmul(pb[:, QS[qh]], dg_sb[:, 1],
                                         aid_sb[:, 2, kt, QS[qh]],
                                         start=False, stop=True)
                    # r = relu(group 1)  [ACT] -> bf16
                    r_t = r_pool.tile([128, S], bf16, tag="rt")
                    rts[kt] = r_t
                    nc.scalar.activation(r_t[:], pb[:], AF.Relu)

                def stage_mid(kt):
                    # group 2, same banks: plain p
                    kqL = kq_sb[:, 0, kt * 128:(kt + 1) * 128]
                    pb, r_t = pbs[kt], rts[kt]
                    for qh in range(QH):
                        nc.tensor.matmul(pb[:, QS[qh]], kqL,
                                         kq_sb[:, 1, QS[qh]],
                                         start=True, stop=True)
                    # pre = sgn*r + p, fused into the PSUM->SBUF move [DVE]
                    pre_t = pre_pool.tile([128, S], f32, tag="pre")
                    nc.vector.scalar_tensor_tensor(
                        pre_t[:], r_t[:], par_sb[:, 4 + pair:5 + pair],
                        pb[:], OP.mult, OP.add)
                    nc.sync.dma_start(
                        pre_d[pair, kt * 128:(kt + 1) * 128, :], pre_t[:])
                    # e = exp(pre) [ACT] -> bf16
                    nc.scalar.activation(e_sb[:, kt], pre_t[:], AF.Exp)

                def stage_back(kt):
                    # out2T rows 0:64 += v.T-free @ e ; Z row 64 via the
                    # ones column folded into vx.
                    for qh in range(QH):
                        nc.tensor.matmul(o2z[0:HD + 1, QS[qh]], vx_sb[:, kt],
                                         e_sb[:, kt, QS[qh]],
                                         start=(kt == 0), stop=(kt == KT - 1))

                for kt in range(KT + 2):
                    if kt < KT:
                        stage_front(kt)
                    if 1 <= kt:
                        if kt - 1 < KT:
                            stage_mid(kt - 1)
                    if kt >= 2:
                        stage_back(kt - 2)

                # ---- pair tail ----
                # 1/Z: spread the Z row across 128 partitions via a DRAM
                # bounce so the (slow, iterative) reciprocal runs on 8
                # elements/lane instead of 1024 on one lane.
                zrow = srow_pool.tile([KD, S], f32, tag="zrow")
                nc.vector.tensor_copy(zrow[HD:HD + 1, :], o2z[HD:HD + 1, :])
                nc.sync.dma_start(zs_d[pair], zrow[HD:HD + 1, :])
                spread = srow_pool.tile([128, S // 128], f32, tag="spread")
                nc.sync.dma_start(
                    spread[:], zs_d[pair].rearrange("(p f) -> p f", p=128))
                rec128 = srow_pool.tile([128, S // 128], f32, tag="rec128")
                nc.vector.reciprocal(rec128[:], spread[:])
                nc.sync.dma_start(
                    zs2_d[pair].rearrange("(p f) -> p f", p=128), rec128[:])
                # partition_broadcast reads absolute partition 0, so land the
                # full recip row there and broadcast.
                srow0 = srow_pool.tile([1, S], f32, tag="srow0")
                nc.sync.dma_start(
                    srow0[0:1, :], zs2_d[pair].rearrange("(a f) -> a f", a=1))
                zb = zb_pool.tile([128, S], f32, tag="zb")
                nc.gpsimd.partition_broadcast(zb[:], srow0[0:1, :], 128)

                o2sb = o2sb_pool.tile([128, S], f32, tag="o2sb")
                nc.scalar.copy(o2sb[0:HD, :], o2z[0:HD, :])
                if trow == 0:
                    nc.vector.tensor_tensor(
                        actT_sb[0:HD, tcol], o2sb[0:HD, :], zb[0:HD, :],
                        OP.mult)
                else:
                    o2n = o2sb_pool.tile([HD, S], bf16, tag="o2n")
                    nc.vector.tensor_tensor(
                        o2n[:], o2sb[0:HD, :], zb[0:HD, :], OP.mult)
                    nc.sync.dma_start(actT_sb[trow:trow + HD, tcol], o2n[:])

                for kt in range(KT):
                    attn_t = attn_pool.tile([128, S], f32, tag="att")
                    eng = nc.vector if kt % 2 == 0 else nc.gpsimd
                    eng.tensor_tensor(attn_t[:], e_sb[:, kt], zb[:], OP.mult)
                    nc.sync.dma_start(
                        attn_d[pair, kt * 128:(kt + 1) * 128, :], attn_t[:])

            # ---- row-parallel projection: finT = WoT.T @ actT ----
            for eo in range(8):
                es = slice(eo * 128, (eo + 1) * 128)
                for qh in range(QH):
                    qs = slice(qh * 512, (qh + 1) * 512)
                    fin_ps = ppb_pool.tile([128, 512], f32, tag="ppb")
                    nc.tensor.matmul(fin_ps[:], wot_sb[:, 0, es],
                                     actT_sb[:, 0, qs], start=True, stop=False)
                    nc.tensor.matmul(fin_ps[:], wot_sb[:, 1, es],
                                     actT_sb[:, 1, qs], start=False, stop=True)
                    fin_t = fin_pool.tile([128, 512], f32, tag="fin")
                    if eo % 2 == 0:
                        nc.vector.tensor_copy(fin_t[:], fin_ps[:])
                    else:
                        nc.scalar.copy(fin_t[:], fin_ps[:])
                    nc.sync.dma_start(fin_d[es, qs], fin_t[:])

    nc.compile()
    return nc


def get_program():
    if "nc" not in _PROG_CACHE:
        _PROG_CACHE["nc"] = _build_program()
    return _PROG_CACHE["nc"]


def make_core_inputs(ci, query, key, value, aid, mixer_w, mixer_b, Wo, bo,
                     aid_scale):
    """Build the per-core input map (numpy, host-side sharding/layout)."""
    b = ci // 4
    heads = [4 * (ci % 4) + j for j in range(NPAIR)]
    c = float(np.asarray(aid_scale).reshape(-1)[0])
    a_abs = abs(c)
    sgn = 1.0 if c >= 0 else -1.0

    q4 = query.reshape(B, H, S, HD)
    k4 = key.reshape(B, H, S, HD)
    v4 = value.reshape(B, H, S, HD)

    # aid_l[p, ch, kt, q] = aid[b, q, kt*128+p, ch]
    aid_l = np.ascontiguousarray(
        aid[b].transpose(2, 1, 0)          # (ch, k, q)
        .reshape(A, KT, 128, S)
        .transpose(2, 0, 1, 3)
    ).astype(BF16)

    kq_l = np.zeros((KD, NPAIR, 3, S), np.float32)
    vx_l = np.zeros((128, NPAIR, KT, HD + 1), np.float32)
    vx_l[:, :, :, HD] = 1.0
    dg_l = np.zeros((128, NPAIR, 3, 128), np.float32)
    par_l = np.zeros((128, 8), np.float32)

    eye = np.eye(128, dtype=np.float32)
    for j, h in enumerate(heads):
        w0, w1, w2, w3 = (float(mixer_w[h, t, 0]) for t in range(A + 1))
        bm = float(mixer_b[h, 0])
        w2s = w2 if abs(w2) > 1e-8 else 1e-8
        qT = q4[b, h].T                    # (HD, S)
        kT = k4[b, h].T
        kq_l[:HD, j, 0] = kT
        kq_l[HD, j, 0] = 1.0
        kq_l[:HD, j, 1] = SCALE * qT
        kq_l[HD, j, 1] = 0.0
        kq_l[:HD, j, 2] = (a_abs * w0 * SCALE) * qT
        kq_l[HD, j, 2] = a_abs * bm
        vx_l[:, j, :, :HD] = v4[b, h].reshape(KT, 128, HD).transpose(1, 0, 2)
        dg_l[:, j, 0] = (a_abs * w2s) * eye
        dg_l[:, j, 1] = (a_abs * w3) * eye
        dg_l[:, j, 2] = sgn * eye
        par_l[:, j] = w1 / w2s
        par_l[:, 4 + j] = sgn

    e0 = 256 * (ci % 4)
    # wot_l[p, t, e_out] = Wo[e_out, e0 + t*128 + p]
    wot_l = np.ascontiguousarray(
        Wo[:, e0:e0 + 256].T.reshape(2, 128, E).transpose(1, 0, 2)
    ).astype(BF16)

    return {
        "aid_l": aid_l,
        "kq_l": kq_l.astype(BF16),
        "vx_l": vx_l.astype(BF16),
        "dg_l": dg_l.astype(BF16),
        "wot_l": wot_l,
        "par_l": par_l,
    }


def _enable_ldw_opt():
    """Walrus's LDWEIGHTS-dedup pass is disabled by default in
    bir_verify_and_optimise; our matmul stream re-loads identical
    stationary weights constantly, so turn it on (correctness is
    covered by the caller's rel-err check)."""
    return  # walrus crashes with ldw-opt=true; keep default
    if _PROG_CACHE.get("ldw_patched"):
        return
    import concourse.bass_utils as bu

    orig = bu.run_command

    def patched(argv, **kw):
        argv = [a.replace("--enable-ldw-opt=false", "--enable-ldw-opt=true")
                if isinstance(a, str) else a for a in argv]
        return orig(argv, **kw)

    bu.run_command = patched
    _PROG_CACHE["ldw_patched"] = True


def kernel(query, key, value, aid, mixer_w, mixer_b, Wo, bo, aid_scale,
           trace=False):
    from concourse.bass_utils import run_bass_kernel_spmd

    _enable_ldw_opt()

    query = np.asarray(query, np.float32)
    key = np.asarray(key, np.float32)
    value = np.asarray(value, np.float32)
    aid = np.asarray(aid, np.float32)
    mixer_w = np.asarray(mixer_w, np.float32)
    mixer_b = np.asarray(mixer_b, np.float32)
    Wo = np.asarray(Wo, np.float32)
    bo = np.asarray(bo, np.float32)
    aid_scale = np.asarray(aid_scale, np.float32)

    nc = get_program()
    in_maps = [
        make_core_inputs(ci, query, key, value, aid, mixer_w, mixer_b, Wo,
                         bo, aid_scale)
        for ci in range(NCORES)
    ]
    res = run_bass_kernel_spmd(nc, in_maps, list(range(NCORES)), trace=trace)
    _PROG_CACHE["last_results"] = res

    out = np.empty((B, S, E), np.float32)
    attn = np.empty((B, H, S, S), np.float32)
    pre = np.empty((B, H, S, S), np.float32)
    fin_acc = [None] * B
    for ci in range(NCORES):
        b = ci // 4
        r = res.results[ci]
        for j in range(NPAIR):
            h = 4 * (ci % 4) + j
            attn[b, h] = r["attn_o"][j].T
            pre[b, h] = r["pre_o"][j].T
        fin_acc[b] = r["fin_o"] if fin_acc[b] is None else fin_acc[b] + r["fin_o"]
    for b in range(B):
        out[b] = fin_acc[b].T + bo[None, :]
    return out, attn, pre
